# revision 1
# baseline (speedup 1.0000x reference)
"""Class-conditional linear dispatch (MoE routing) on 8 trn2 NeuronCores.

y[i] = x[i] @ W[cls[i]] + b[cls[i]]   with B=8192, D=512, C=16 classes.

Strategy: expert-parallel. Host computes the per-class row lists (the
routing / all-to-all dispatch) from `cls`; core k owns classes {2k, 2k+1}.
Each core receives the full x (replicated), its 2 weight matrices + biases,
and an int16 row-index list. On device, rows are gathered with
gpsimd.dma_gather (SWDGE indirect DMA), transposed 128x128 on the PE, then
multiplied against the per-class weights (float32r matmuls, K accumulated in
PSUM), bias-added on DVE, and written back densely in class-sorted order.
The host scatters the compact per-core outputs back to original row order.
"""

import os
import sys

import numpy as np

_TRN_REPO = "/opt/trn_rl_repo"
if _TRN_REPO not in sys.path:
    sys.path.insert(0, _TRN_REPO)

B, D_IN, D_OUT, C, NCORES = 8192, 512, 512, 16, 8
CPL = C // NCORES  # classes per core
KC = D_IN // 128  # contraction chunks of 128

# Set by callers that want profiling; results stashed in LAST_RESULT.
TRACE = False
LAST_RESULT = None


def plan_chunks(n_tiles, gather_chunk, first_small, c):
    """Row-tile chunk sizes for class c's gathers (must match host & device)."""
    chunks, rem = [], n_tiles
    if first_small and c == 0 and n_tiles > 1:
        chunks.append(1)
        rem -= 1
    while rem > 0:
        gc = min(gather_chunk, rem)
        chunks.append(gc)
        rem -= gc
    return chunks


def build_nc(
    s_cap: int,
    *,
    w_batch: bool = False,
    out_batch: bool = False,
    gather_chunk: int = 1,
    gbufs: int = 4,
    reps: int = 1,
    loop_reps: int = 1,
    swdge_queues: int = 1,
    first_small: bool = False,
    w_defer: bool = False,
    skip_pad: bool = False,
    store_act: bool = False,
    deep_bufs: bool = False,
):
    """Build + compile the per-core Bass program for class capacity s_cap
    (rows per class, multiple of 128).

    w_batch: load each class's W as one 1 MiB DMA instead of 4x 256 KiB.
    out_batch: accumulate each class's outputs in SBUF, write once per class.
    gather_chunk: row-tiles per dma_gather call (1 = per-tile gathers).
    reps: repeat the whole computation (hardware-loop-free) for wall-clock
          benchmarking; outputs are overwritten identically each rep.
    """
    import concourse.bacc as bacc
    import concourse.mybir as mybir
    from concourse import tile

    f32 = mybir.dt.float32
    f32r = mybir.dt.float32r
    i16 = mybir.dt.int16
    n_tiles = s_cap // 128
    r_cap = CPL * s_cap

    nc = bacc.Bacc(
        "TRN2",
        target_bir_lowering=False,
        debug=False,
        num_swdge_queues=swdge_queues,
    )
    x_d = nc.dram_tensor("x", [B, D_IN], f32r, kind="ExternalInput")
    idx_d = nc.dram_tensor("idx", [128, r_cap // 16], i16, kind="ExternalInput")
    w_d = nc.dram_tensor("wl", [CPL, D_IN, D_OUT], f32r, kind="ExternalInput")
    b_d = nc.dram_tensor("bl", [1, CPL * D_OUT], f32, kind="ExternalInput")
    id_d = nc.dram_tensor("ident", [128, 128], f32r, kind="ExternalInput")
    n_chunks_total = CPL * len(plan_chunks(n_tiles, gather_chunk, first_small, 0))
    if skip_pad:
        n_chunks_total = sum(
            len(plan_chunks(n_tiles, gather_chunk, first_small, c))
            for c in range(CPL)
        )
        cnt_d = nc.dram_tensor(
            "cnt", [1, max(1, n_chunks_total)], mybir.dt.int32, kind="ExternalInput"
        )
    y_d = nc.dram_tensor("y", [r_cap, D_OUT], f32, kind="ExternalOutput")

    with tile.TileContext(nc) as tc:
        from contextlib import nullcontext

        with (
            tc.tile_pool(name="const", bufs=1) as cpool,
            tc.tile_pool(name="gather", bufs=gbufs) as gpool,
            tc.tile_pool(name="xt", bufs=4 if deep_bufs else 3) as xtpool,
            tc.tile_pool(
                name="yout", bufs=2 if out_batch else (4 if deep_bufs else 3)
            ) as ypool,
            tc.tile_pool(name="pst", bufs=3 if deep_bufs else 2, space="PSUM") as pstp,
            tc.tile_pool(name="psy", bufs=3 if deep_bufs else 2, space="PSUM") as psyp,
            tc.For_i(0, loop_reps, 1) if loop_reps > 1 else nullcontext(),
        ):
            for _ in range(reps):
                # issue order matters: small index/ident loads first, then
                # interleave gathers and W loads per class so the first
                # class's compute can start while the second class streams in.
                idx_sb = cpool.tile([128, r_cap // 16], i16, tag="idx")
                nc.sync.dma_start(idx_sb[:], idx_d[:])
                ident = cpool.tile([128, 128], f32r, tag="ident")
                nc.sync.dma_start(ident[:], id_d[:])

                if skip_pad:
                    cnt_sb = cpool.tile(
                        [1, max(1, n_chunks_total)], mybir.dt.int32, tag="cnt"
                    )
                    nc.sync.dma_start(cnt_sb[:1, :], cnt_d[:1, :])
                    cnt_reg = nc.gpsimd.alloc_register("gcnt")

                w_sb = cpool.tile([128, CPL * KC, D_OUT], f32r, tag="w")
                tile_src = {}  # (c, t) -> (gather tile, slot)
                first_gather_inst = None
                n_gather = 0
                for c in range(CPL):
                    t0 = 0
                    for gc in plan_chunks(n_tiles, gather_chunk, first_small, c):
                        seg = c * n_tiles + t0
                        g = gpool.tile([128, gather_chunk, D_IN], f32r)
                        if skip_pad:
                            nc.gpsimd.reg_load(
                                cnt_reg, cnt_sb[:1, n_gather : n_gather + 1]
                            )
                            nreg = cnt_reg
                        else:
                            nreg = gc * 128
                        gi = nc.gpsimd.dma_gather(
                            g[:, :gc, :],
                            x_d[:],
                            idx_sb[:, seg * 8 : (seg + gc) * 8],
                            gc * 128,
                            nreg,
                            D_IN,
                            queue_num=n_gather % swdge_queues,
                        )
                        if first_gather_inst is None:
                            first_gather_inst = gi
                        for j in range(gc):
                            tile_src[(c, t0 + j)] = (g, j)
                        t0 += gc
                        n_gather += 1
                    # W[c] stored K-chunked after this class's gathers
                    if w_batch:
                        wi = nc.sync.dma_start(
                            w_sb[:, c * KC : (c + 1) * KC, :],
                            w_d[c].rearrange("(kc p) n -> p kc n", p=128),
                        )
                        if w_defer and c > 0 and first_gather_inst is not None:
                            tile.add_dep_helper(
                                wi.ins,
                                first_gather_inst.ins,
                                reason="defer late-class W behind first gather",
                            )
                    else:
                        for k in range(KC):
                            nc.sync.dma_start(
                                w_sb[:, c * KC + k, :],
                                w_d[c, k * 128 : (k + 1) * 128, :],
                            )

                # bias, replicated across all partitions (needed late)
                b_row = cpool.tile([1, CPL * D_OUT], f32, tag="brow")
                nc.sync.dma_start(b_row[:1, :], b_d[:1, :])
                b_bc = cpool.tile([128, CPL, D_OUT], f32, tag="bbc")
                nc.gpsimd.partition_broadcast(b_bc[:], b_row[:1, :])

                for c in range(CPL):
                    if out_batch:
                        y_big = ypool.tile([128, n_tiles, D_OUT], f32)
                    for t in range(n_tiles):
                        g, gslot = tile_src[(c, t)]
                        xt_ps = pstp.tile([128, D_IN], f32r)
                        for k in range(KC):
                            nc.tensor.transpose(
                                xt_ps[:, k * 128 : (k + 1) * 128],
                                g[:, gslot, k * 128 : (k + 1) * 128],
                                ident[:],
                            )
                        xt = xtpool.tile([128, KC, 128], f32r)
                        nc.vector.tensor_copy(xt[:], xt_ps[:])
                        y_ps = psyp.tile([128, D_OUT], f32)
                        for k in range(KC):
                            nc.tensor.matmul(
                                y_ps[:],
                                xt[:, k, :],
                                w_sb[:, c * KC + k, :],
                                start=(k == 0),
                                stop=(k == KC - 1),
                            )
                        if out_batch:
                            nc.vector.tensor_add(
                                y_big[:, t, :], y_ps[:], b_bc[:, c, :]
                            )
                        else:
                            y_sb = ypool.tile([128, D_OUT], f32)
                            nc.vector.tensor_add(y_sb[:], y_ps[:], b_bc[:, c, :])
                            row0 = c * s_cap + t * 128
                            store_eng = nc.scalar if store_act else nc.sync
                            store_eng.dma_start(y_d[row0 : row0 + 128, :], y_sb[:])
                    if out_batch:
                        nc.sync.dma_start(
                            y_d[c * s_cap : (c + 1) * s_cap, :].rearrange(
                                "(t p) n -> p t n", p=128
                            ),
                            y_big[:],
                        )

    nc.compile()
    return nc


def _route(cls_np: np.ndarray):
    """Per-class row lists + uniform per-class capacity (multiple of 128)."""
    order = np.argsort(cls_np, kind="stable")
    counts = np.bincount(cls_np, minlength=C)
    starts = np.zeros(C + 1, dtype=np.int64)
    starts[1:] = np.cumsum(counts)
    rows_per_class = [order[starts[c] : starts[c + 1]] for c in range(C)]
    s_cap = max(128, int(-(-int(counts.max()) // 128)) * 128)
    return rows_per_class, s_cap


# Variant shipped by kernel(); bench.py sweeps alternatives.
BEST_VARIANT = {"w_batch": True, "gather_chunk": 3, "first_small": True}


def make_in_maps(x, rows_per_class, W, b, s_cap, **variant):
    """Per-core input maps matching build_nc(s_cap, **variant)."""
    r_cap = CPL * s_cap
    n_tiles = s_cap // 128
    skip_pad = variant.get("skip_pad", False)
    gather_chunk = variant.get("gather_chunk", 1)
    first_small = variant.get("first_small", False)
    in_maps = []
    for k in range(NCORES):
        fill = -1 if skip_pad else 0
        idx_full = np.full(r_cap, fill, dtype=np.int64)
        for j in range(CPL):
            rows = rows_per_class[CPL * k + j]
            idx_full[j * s_cap : j * s_cap + len(rows)] = rows
        cnts = []
        if skip_pad:
            for c in range(CPL):
                t0 = 0
                for gc in plan_chunks(n_tiles, gather_chunk, first_small, c):
                    lo = c * s_cap + t0 * 128
                    hi = lo + gc * 128
                    valid = int((idx_full[lo:hi] >= 0).sum())
                    if valid == 0:
                        idx_full[lo] = 0  # keep >=1 valid index per gather
                        valid = 1
                    cnts.append(valid)
                    t0 += gc
        idx2d = np.tile(idx_full.reshape(-1, 16).T.astype(np.int16), (8, 1))
        m = {
            "x": np.ascontiguousarray(x, dtype=np.float32),
            "idx": np.ascontiguousarray(idx2d),
            "wl": np.ascontiguousarray(W[CPL * k : CPL * (k + 1)]),
            "bl": np.ascontiguousarray(
                b[CPL * k : CPL * (k + 1)].reshape(1, CPL * D_OUT)
            ),
            "ident": np.eye(128, dtype=np.float32),
        }
        if skip_pad:
            m["cnt"] = np.asarray([cnts], dtype=np.int32)
        in_maps.append(m)
    return in_maps


def kernel(x, cls, W, b):
    from concourse.bass_utils import run_bass_kernel_spmd

    global LAST_RESULT
    x = np.ascontiguousarray(np.asarray(x), dtype=np.float32)
    cls_np = np.asarray(cls).astype(np.int64).ravel()
    W = np.ascontiguousarray(np.asarray(W), dtype=np.float32)
    b = np.ascontiguousarray(np.asarray(b), dtype=np.float32)

    rows_per_class, s_cap = _route(cls_np)

    in_maps = make_in_maps(x, rows_per_class, W, b, s_cap, **BEST_VARIANT)
    nc = build_nc(s_cap, **BEST_VARIANT)
    res = run_bass_kernel_spmd(
        nc,
        in_maps,
        core_ids=list(range(NCORES)),
        trace=TRACE,
        trace_cores=list(range(NCORES)) if TRACE else None,
    )
    LAST_RESULT = res

    out = np.empty((B, D_OUT), dtype=np.float32)
    for k in range(NCORES):
        y = res.results[k]["y"]
        for j in range(CPL):
            rows = rows_per_class[CPL * k + j]
            out[rows] = y[j * s_cap : j * s_cap + len(rows)]
    return out

